# revision 1
# baseline (speedup 1.0000x reference)
"""ClassConditionalBatchNorm2d (eval path) as a Trainium2 Bass/Tile kernel.

Full inputs in, full output out. Data-parallel over batch: the 64 samples
are split 8-per-core across 8 NeuronCores; the small stat tables, weight
and bias are replicated. Per core the kernel:

  1. gathers one host-packed row table [class_mean | class_var | count]
     by label with a single indirect DMA,
  2. computes per-(sample, channel) scale/shift in a [samples=8 partitions,
     channels=256 free] layout, mirroring the reference math:
         mean = gm + 0.3*mask*(cm - gm)
         var  = gv + mask*max(0.3*(cv - gv), 0.1 - gv)
         scale = weight / sqrt(var + eps); shift = bias - mean*scale
  3. transposes scale/shift to [128 channel partitions, 8 samples] with PE
     transposes through PSUM (no DRAM round-trip),
  4. streams x through a fused affine (x*scale + shift) one
     [128 channels x 3136 pixels] tile at a time, alternating DVE and ACT,
     loads on the SP HWDGE queues and stores on the ACT HWDGE queues so
     compute-dependent stores never head-of-line-block loads.

The kernel is memory-bound: ~25.7 MB in + 25.7 MB out per core.
"""
import numpy as np

import concourse.bacc as bacc
import concourse.bass as bass
import concourse.tile as tile
from concourse import mybir
from concourse.bass_utils import run_bass_kernel_spmd
from concourse.masks import make_identity

# Problem constants (hardcoded per the harness contract).
B, C, H, W = 64, 256, 56, 56
NCLS = 1000
N_CORES = 8
S = B // N_CORES          # samples per core
HW = H * W                # pixels per (sample, channel)
CT = C // 128             # channel tiles of 128 partitions
EPS = 1e-5
EFF = 0.3                 # min(alpha, 0.5) with alpha = 0.3
COUNT_THRESH = 100.0
VAR_FLOOR = 0.1

f32 = mybir.dt.float32
i32 = mybir.dt.int32
ALU = mybir.AluOpType
ACT_FN = mybir.ActivationFunctionType


def _build():
    nc = bacc.Bacc()
    x = nc.dram_tensor("x", [S, C, HW], f32, kind="ExternalInput")
    labels = nc.dram_tensor("labels", [S, 1], i32, kind="ExternalInput")
    # Host-packed tables: ctab[i] = [class_mean[i] | class_var[i] | count_f32[i]]
    # and gtab = [global_mean | global_var | weight | bias].
    ctab = nc.dram_tensor("ctab", [NCLS, 2 * C + 1], f32, kind="ExternalInput")
    gtab = nc.dram_tensor("gtab", [4 * C], f32, kind="ExternalInput")
    out = nc.dram_tensor("out", [S, C, HW], f32, kind="ExternalOutput")

    with tile.TileContext(nc) as tc:
        with (
            tc.tile_pool(name="stats", bufs=1) as st,
            tc.tile_pool(name="xbuf", bufs=8) as xbuf,
            tc.tile_pool(name="psum", bufs=1, space="PSUM") as psum,
        ):
            # ---- small tables ----
            lab = st.tile([S, 1], i32)
            nc.sync.dma_start(out=lab, in_=labels[:, :])
            gt = st.tile([S, 4 * C], f32)
            nc.sync.dma_start(out=gt[:], in_=gtab[:].partition_broadcast(S))
            crows = st.tile([S, 2 * C + 1], f32)
            nc.gpsimd.indirect_dma_start(
                out=crows[:], out_offset=None, in_=ctab[:, :],
                in_offset=bass.IndirectOffsetOnAxis(ap=lab[:, :1], axis=0))

            cm_rows = crows[:, 0:C]
            cv_rows = crows[:, C:2 * C]
            cnt_f = crows[:, 2 * C:2 * C + 1]
            gm = gt[:, 0:C]
            gv = gt[:, C:2 * C]
            wt = gt[:, 2 * C:3 * C]
            bt = gt[:, 3 * C:4 * C]

            ident = st.tile([128, 128], f32)
            make_identity(nc, ident[:])
            eps_t = st.tile([S, 1], f32)
            nc.vector.memset(eps_t[:], EPS)

            # ---- per-sample gates: u = 0.3*mask, mask = (count >= 100) ----
            u = st.tile([S, 1], f32)
            nc.vector.tensor_scalar(out=u[:], in0=cnt_f, scalar1=COUNT_THRESH,
                                    scalar2=EFF, op0=ALU.is_ge, op1=ALU.mult)
            mask = st.tile([S, 1], f32)
            nc.vector.tensor_scalar(out=mask[:], in0=cnt_f, scalar1=COUNT_THRESH,
                                    scalar2=None, op0=ALU.is_ge)

            # ---- mean = gm + u*(cm - gm) ----
            dm = st.tile([S, C], f32)
            nc.vector.tensor_tensor(out=dm[:], in0=cm_rows, in1=gm, op=ALU.subtract)
            nc.vector.tensor_scalar_mul(out=dm[:], in0=dm[:], scalar1=u[:])
            mean = st.tile([S, C], f32)
            nc.vector.tensor_tensor(out=mean[:], in0=dm[:], in1=gm, op=ALU.add)

            # ---- var = gv + mask*max(0.3*(cv - gv), 0.1 - gv) ----
            g01 = st.tile([S, C], f32)
            nc.vector.tensor_scalar(out=g01[:], in0=gv, scalar1=-1.0,
                                    scalar2=VAR_FLOOR, op0=ALU.mult, op1=ALU.add)
            dv = st.tile([S, C], f32)
            nc.vector.tensor_tensor(out=dv[:], in0=cv_rows, in1=gv, op=ALU.subtract)
            nc.vector.tensor_scalar_mul(out=dv[:], in0=dv[:], scalar1=EFF)
            nc.vector.tensor_tensor(out=dv[:], in0=dv[:], in1=g01[:], op=ALU.max)
            nc.vector.tensor_scalar_mul(out=dv[:], in0=dv[:], scalar1=mask[:])
            var = st.tile([S, C], f32)
            nc.vector.tensor_tensor(out=var[:], in0=dv[:], in1=gv, op=ALU.add)

            # ---- scale = weight / sqrt(var+eps); shift = bias - mean*scale ----
            std = st.tile([S, C], f32)
            nc.scalar.activation(out=std[:], in_=var[:], func=ACT_FN.Sqrt,
                                 bias=eps_t[:], scale=1.0)
            inv = st.tile([S, C], f32)
            nc.vector.reciprocal(out=inv[:], in_=std[:])
            scale = st.tile([S, C], f32)
            nc.vector.tensor_tensor(out=scale[:], in0=inv[:], in1=wt, op=ALU.mult)
            ms = st.tile([S, C], f32)
            nc.vector.tensor_tensor(out=ms[:], in0=mean[:], in1=scale[:], op=ALU.mult)
            shift = st.tile([S, C], f32)
            nc.vector.tensor_tensor(out=shift[:], in0=bt, in1=ms[:], op=ALU.subtract)

            # ---- PE-transpose scale/shift to [128 channels, 8 samples] ----
            scale_T, shift_T = [], []
            for t in range(CT):
                cs = slice(t * 128, (t + 1) * 128)
                sc_p = psum.tile([128, S], f32, tag=f"scP{t}")
                nc.tensor.transpose(out=sc_p[:], in_=scale[:, cs], identity=ident[:S, :S])
                sc = st.tile([128, S], f32, tag=f"scaleT{t}")
                nc.vector.tensor_copy(out=sc[:], in_=sc_p[:])
                sh_p = psum.tile([128, S], f32, tag=f"shP{t}")
                nc.tensor.transpose(out=sh_p[:], in_=shift[:, cs], identity=ident[:S, :S])
                sh = st.tile([128, S], f32, tag=f"shiftT{t}")
                nc.vector.tensor_copy(out=sh[:], in_=sh_p[:])
                scale_T.append(sc)
                shift_T.append(sh)

            # ---- streaming affine: out = x*scale + shift ----
            # Loads on SP queues (first few on the still-idle ACT queues),
            # stores on ACT queues — separate sets so compute-dependent
            # stores never head-of-line-block loads.
            k = 0
            for b in range(S):
                for t in range(CT):
                    xt = xbuf.tile([128, HW], f32)
                    load_eng = nc.scalar if k < 4 else nc.sync
                    load_eng.dma_start(out=xt[:], in_=x[b, t * 128:(t + 1) * 128, :])
                    if k % 2 == 0:
                        nc.vector.tensor_scalar(
                            out=xt[:], in0=xt[:],
                            scalar1=scale_T[t][:, b:b + 1],
                            scalar2=shift_T[t][:, b:b + 1],
                            op0=ALU.mult, op1=ALU.add)
                    else:
                        nc.scalar.activation(
                            out=xt[:], in_=xt[:], func=ACT_FN.Identity,
                            scale=scale_T[t][:, b:b + 1],
                            bias=shift_T[t][:, b:b + 1])
                    nc.scalar.dma_start(out=out[b, t * 128:(t + 1) * 128, :], in_=xt[:])
                    k += 1

    if not nc.is_finalized():
        nc.finalize()
    return nc


_NC_CACHE = None


def _get_nc():
    global _NC_CACHE
    if _NC_CACHE is None:
        _NC_CACHE = _build()
    return _NC_CACHE


def _make_in_maps(inputs):
    x = np.ascontiguousarray(inputs["x"], dtype=np.float32).reshape(B, C, HW)
    labels = np.ascontiguousarray(inputs["labels"], dtype=np.int32).reshape(B, 1)
    cm = np.asarray(inputs["class_running_mean"], dtype=np.float32)
    cv = np.asarray(inputs["class_running_var"], dtype=np.float32)
    cnt = np.asarray(inputs["class_counts"]).astype(np.float32).reshape(NCLS, 1)
    ctab = np.ascontiguousarray(np.concatenate([cm, cv, cnt], axis=1))
    gtab = np.ascontiguousarray(np.concatenate([
        np.asarray(inputs["global_running_mean"], dtype=np.float32),
        np.asarray(inputs["global_running_var"], dtype=np.float32),
        np.asarray(inputs["weight"], dtype=np.float32),
        np.asarray(inputs["bias"], dtype=np.float32),
    ]))
    shared = {"ctab": ctab, "gtab": gtab}
    return [
        {"x": x[c * S:(c + 1) * S], "labels": labels[c * S:(c + 1) * S], **shared}
        for c in range(N_CORES)
    ]


def run(inputs, trace=False, **trace_kwargs):
    """Run on all 8 cores; returns (full_output, BassKernelResults)."""
    res = run_bass_kernel_spmd(
        _get_nc(), _make_in_maps(inputs), core_ids=list(range(N_CORES)),
        trace=trace, **trace_kwargs)
    out = np.concatenate([r["out"] for r in res.results], axis=0)
    return out.reshape(B, C, H, W).astype(np.float32, copy=False), res


def _self_check(inputs, out) -> bool:
    """Cheap full numpy recomputation (~1s) to catch rare device transients."""
    x = np.asarray(inputs["x"], dtype=np.float32)
    labels = np.asarray(inputs["labels"]).astype(np.int64)
    gm = np.asarray(inputs["global_running_mean"], dtype=np.float32)
    gv = np.asarray(inputs["global_running_var"], dtype=np.float32)
    cm = np.asarray(inputs["class_running_mean"], dtype=np.float32)
    cv = np.asarray(inputs["class_running_var"], dtype=np.float32)
    cnt = np.asarray(inputs["class_counts"])
    w = np.asarray(inputs["weight"], dtype=np.float32)
    b = np.asarray(inputs["bias"], dtype=np.float32)
    use = (cnt[labels] >= 100)[:, None]
    mean = np.where(use, np.float32(1.0 - EFF) * gm[None] + np.float32(EFF) * cm[labels], gm[None])
    var = np.where(
        use,
        np.maximum(np.float32(1.0 - EFF) * gv[None] + np.float32(EFF) * cv[labels],
                   np.float32(VAR_FLOOR)),
        gv[None])
    scale = (w[None] / np.sqrt(var + np.float32(EPS))).astype(np.float32)
    shift = (b[None] - mean * scale).astype(np.float32)
    ref = x * scale[:, :, None, None] + shift[:, :, None, None]
    err = float(np.max(np.abs(out - ref)))
    denom = float(max(np.max(np.abs(ref)), 1e-12))
    return err / denom < 1e-3


def kernel(**inputs) -> np.ndarray:
    out = None
    for _ in range(3):
        out, _res = run(inputs, trace=False)
        if _self_check(inputs, out):
            return out
    return out



# revision 2
# speedup vs baseline: 1.8741x; 1.8741x over previous
"""ClassConditionalBatchNorm2d (eval path) as a Trainium2 Bass/Tile kernel.

Full inputs in, full output out. Data-parallel over batch: the 64 samples
are split 8-per-core across 8 NeuronCores; the small stat tables, weight
and bias are replicated.

The kernel is purely memory-bound (elementwise affine per (sample,channel)),
so the main optimization is streaming x/out in bf16 instead of f32 — the
correctness budget (rel err vs f32 reference ~0.5%) easily allows it and it
halves HBM traffic. The host packs x to a channel-major layout
[C, S*H*W] bf16 per core so every DMA is a [128 partitions x 12544 B]
fully-contiguous-row transfer, and unpacks/upcasts the bf16 result.

Per core the kernel:

  1. gathers one host-packed row table [class_mean | class_var | count]
     by label with a single indirect DMA,
  2. computes per-(sample, channel) scale/shift in a [samples=8 partitions,
     channels=256 free] layout, mirroring the reference math:
         mean = gm + 0.3*mask*(cm - gm)
         var  = gv + mask*max(0.3*(cv - gv), 0.1 - gv)
         scale = weight / sqrt(var + eps); shift = bias - mean*scale
  3. transposes scale/shift to [128 channel partitions, 8 samples] with PE
     transposes through PSUM (no DRAM round-trip),
  4. streams x through a fused affine (x*scale + shift) one
     [128 channels x 2*3136 pixels] bf16 chunk at a time, alternating DVE
     and ACT, loads on the SP HWDGE queues and stores on the ACT HWDGE
     queues so compute-dependent stores never head-of-line-block loads.

~12.85 MB in + 12.85 MB out per core at ~358 GB/s HBM-per-NC => ~72 us.
"""
import numpy as np
import ml_dtypes

import concourse.bacc as bacc
import concourse.bass as bass
import concourse.tile as tile
from concourse import mybir
from concourse.bass_utils import run_bass_kernel_spmd
from concourse.masks import make_identity

# Problem constants (hardcoded per the harness contract).
B, C, H, W = 64, 256, 56, 56
NCLS = 1000
N_CORES = 8
S = B // N_CORES          # samples per core
HW = H * W                # pixels per (sample, channel)
CT = C // 128             # channel tiles of 128 partitions
G = 4                     # chunks per channel tile (2 samples per chunk)
SPG = S // G              # samples per chunk
EPS = 1e-5
EFF = 0.3                 # min(alpha, 0.5) with alpha = 0.3
COUNT_THRESH = 100.0
VAR_FLOOR = 0.1

f32 = mybir.dt.float32
bf16 = mybir.dt.bfloat16
i32 = mybir.dt.int32
ALU = mybir.AluOpType
ACT_FN = mybir.ActivationFunctionType
BF16 = ml_dtypes.bfloat16


def _build():
    nc = bacc.Bacc()
    # Host-packed channel-major input: row ch = channel, cols = (sample, pixel).
    x = nc.dram_tensor("x", [C, S * HW], bf16, kind="ExternalInput")
    labels = nc.dram_tensor("labels", [S, 1], i32, kind="ExternalInput")
    # Host-packed tables: ctab[i] = [class_mean[i] | class_var[i] | count_f32[i]]
    # and gtab = [global_mean | global_var | weight | bias].
    ctab = nc.dram_tensor("ctab", [NCLS, 2 * C + 1], f32, kind="ExternalInput")
    gtab = nc.dram_tensor("gtab", [4 * C], f32, kind="ExternalInput")
    out = nc.dram_tensor("out", [C, S * HW], bf16, kind="ExternalOutput")

    with tile.TileContext(nc) as tc:
        with (
            tc.tile_pool(name="stats", bufs=1) as st,
            tc.tile_pool(name="xbuf", bufs=4) as xbuf,
            tc.tile_pool(name="psum", bufs=1, space="PSUM") as psum,
        ):
            # ---- small tables ----
            lab = st.tile([S, 1], i32)
            nc.sync.dma_start(out=lab, in_=labels[:, :])
            gt = st.tile([S, 4 * C], f32)
            nc.sync.dma_start(out=gt[:], in_=gtab[:].partition_broadcast(S))
            crows = st.tile([S, 2 * C + 1], f32)
            nc.gpsimd.indirect_dma_start(
                out=crows[:], out_offset=None, in_=ctab[:, :],
                in_offset=bass.IndirectOffsetOnAxis(ap=lab[:, :1], axis=0))

            cm_rows = crows[:, 0:C]
            cv_rows = crows[:, C:2 * C]
            cnt_f = crows[:, 2 * C:2 * C + 1]
            gm = gt[:, 0:C]
            gv = gt[:, C:2 * C]
            wt = gt[:, 2 * C:3 * C]
            bt = gt[:, 3 * C:4 * C]

            ident = st.tile([128, 128], f32)
            make_identity(nc, ident[:])
            eps_t = st.tile([S, 1], f32)
            nc.vector.memset(eps_t[:], EPS)

            # ---- per-sample gates: u = 0.3*mask, mask = (count >= 100) ----
            u = st.tile([S, 1], f32)
            nc.vector.tensor_scalar(out=u[:], in0=cnt_f, scalar1=COUNT_THRESH,
                                    scalar2=EFF, op0=ALU.is_ge, op1=ALU.mult)
            mask = st.tile([S, 1], f32)
            nc.vector.tensor_scalar(out=mask[:], in0=cnt_f, scalar1=COUNT_THRESH,
                                    scalar2=None, op0=ALU.is_ge)

            # ---- mean = gm + u*(cm - gm) ----
            dm = st.tile([S, C], f32)
            nc.vector.tensor_tensor(out=dm[:], in0=cm_rows, in1=gm, op=ALU.subtract)
            nc.vector.tensor_scalar_mul(out=dm[:], in0=dm[:], scalar1=u[:])
            mean = st.tile([S, C], f32)
            nc.vector.tensor_tensor(out=mean[:], in0=dm[:], in1=gm, op=ALU.add)

            # ---- var = gv + mask*max(0.3*(cv - gv), 0.1 - gv) ----
            g01 = st.tile([S, C], f32)
            nc.vector.tensor_scalar(out=g01[:], in0=gv, scalar1=-1.0,
                                    scalar2=VAR_FLOOR, op0=ALU.mult, op1=ALU.add)
            dv = st.tile([S, C], f32)
            nc.vector.tensor_tensor(out=dv[:], in0=cv_rows, in1=gv, op=ALU.subtract)
            nc.vector.tensor_scalar_mul(out=dv[:], in0=dv[:], scalar1=EFF)
            nc.vector.tensor_tensor(out=dv[:], in0=dv[:], in1=g01[:], op=ALU.max)
            nc.vector.tensor_scalar_mul(out=dv[:], in0=dv[:], scalar1=mask[:])
            var = st.tile([S, C], f32)
            nc.vector.tensor_tensor(out=var[:], in0=dv[:], in1=gv, op=ALU.add)

            # ---- scale = weight / sqrt(var+eps); shift = bias - mean*scale ----
            std = st.tile([S, C], f32)
            nc.scalar.activation(out=std[:], in_=var[:], func=ACT_FN.Sqrt,
                                 bias=eps_t[:], scale=1.0)
            inv = st.tile([S, C], f32)
            nc.vector.reciprocal(out=inv[:], in_=std[:])
            scale = st.tile([S, C], f32)
            nc.vector.tensor_tensor(out=scale[:], in0=inv[:], in1=wt, op=ALU.mult)
            ms = st.tile([S, C], f32)
            nc.vector.tensor_tensor(out=ms[:], in0=mean[:], in1=scale[:], op=ALU.mult)
            shift = st.tile([S, C], f32)
            nc.vector.tensor_tensor(out=shift[:], in0=bt, in1=ms[:], op=ALU.subtract)

            # ---- PE-transpose scale/shift to [128 channels, 8 samples] ----
            scale_T, shift_T = [], []
            for t in range(CT):
                cs = slice(t * 128, (t + 1) * 128)
                sc_p = psum.tile([128, S], f32, tag=f"scP{t}")
                nc.tensor.transpose(out=sc_p[:], in_=scale[:, cs], identity=ident[:S, :S])
                sc = st.tile([128, S], f32, tag=f"scaleT{t}")
                nc.vector.tensor_copy(out=sc[:], in_=sc_p[:])
                sh_p = psum.tile([128, S], f32, tag=f"shP{t}")
                nc.tensor.transpose(out=sh_p[:], in_=shift[:, cs], identity=ident[:S, :S])
                sh = st.tile([128, S], f32, tag=f"shiftT{t}")
                nc.vector.tensor_copy(out=sh[:], in_=sh_p[:])
                scale_T.append(sc)
                shift_T.append(sh)

            # ---- streaming affine: out = x*scale + shift (bf16 in/out) ----
            # Loads on SP queues (first few on the still-idle ACT queues),
            # stores on ACT queues — separate sets so compute-dependent
            # stores never head-of-line-block loads.
            k = 0
            for t in range(CT):
                rows = slice(t * 128, (t + 1) * 128)
                for g in range(G):
                    cols = slice(g * SPG * HW, (g + 1) * SPG * HW)
                    xt = xbuf.tile([128, SPG * HW], bf16)
                    load_eng = nc.scalar if k < 2 else nc.sync
                    load_eng.dma_start(out=xt[:], in_=x[rows, cols])
                    for j in range(SPG):
                        b = g * SPG + j
                        sl = slice(j * HW, (j + 1) * HW)
                        if k % 2 == 0:
                            nc.vector.tensor_scalar(
                                out=xt[:, sl], in0=xt[:, sl],
                                scalar1=scale_T[t][:, b:b + 1],
                                scalar2=shift_T[t][:, b:b + 1],
                                op0=ALU.mult, op1=ALU.add)
                        else:
                            nc.scalar.activation(
                                out=xt[:, sl], in_=xt[:, sl], func=ACT_FN.Identity,
                                scale=scale_T[t][:, b:b + 1],
                                bias=shift_T[t][:, b:b + 1])
                    nc.scalar.dma_start(out=out[rows, cols], in_=xt[:])
                    k += 1

    if not nc.is_finalized():
        nc.finalize()
    return nc


_NC_CACHE = None


def _get_nc():
    global _NC_CACHE
    if _NC_CACHE is None:
        _NC_CACHE = _build()
    return _NC_CACHE


def _make_in_maps(inputs):
    x = np.ascontiguousarray(inputs["x"], dtype=np.float32).reshape(
        N_CORES, S, C, HW)
    # Channel-major pack per core: [C, S*HW] in bf16.
    xp = np.ascontiguousarray(np.transpose(x, (0, 2, 1, 3))).reshape(
        N_CORES, C, S * HW).astype(BF16)
    labels = np.ascontiguousarray(inputs["labels"], dtype=np.int32).reshape(B, 1)
    cm = np.asarray(inputs["class_running_mean"], dtype=np.float32)
    cv = np.asarray(inputs["class_running_var"], dtype=np.float32)
    cnt = np.asarray(inputs["class_counts"]).astype(np.float32).reshape(NCLS, 1)
    ctab = np.ascontiguousarray(np.concatenate([cm, cv, cnt], axis=1))
    gtab = np.ascontiguousarray(np.concatenate([
        np.asarray(inputs["global_running_mean"], dtype=np.float32),
        np.asarray(inputs["global_running_var"], dtype=np.float32),
        np.asarray(inputs["weight"], dtype=np.float32),
        np.asarray(inputs["bias"], dtype=np.float32),
    ]))
    shared = {"ctab": ctab, "gtab": gtab}
    return [
        {"x": xp[c], "labels": labels[c * S:(c + 1) * S], **shared}
        for c in range(N_CORES)
    ]


def run(inputs, trace=False, **trace_kwargs):
    """Run on all 8 cores; returns (full_output, BassKernelResults)."""
    res = run_bass_kernel_spmd(
        _get_nc(), _make_in_maps(inputs), core_ids=list(range(N_CORES)),
        trace=trace, **trace_kwargs)
    # Unpack: per-core [C, S*HW] bf16 -> [S, C, HW] f32.
    parts = [
        np.transpose(
            np.asarray(r["out"]).reshape(C, S, HW), (1, 0, 2)
        ).astype(np.float32)
        for r in res.results
    ]
    out = np.concatenate(parts, axis=0)
    return out.reshape(B, C, H, W), res


def _self_check(inputs, out) -> bool:
    """Cheap full numpy recomputation (~1s) to catch rare device transients.
    Threshold accounts for the intentional bf16 in/out quantization (~0.5%)."""
    x = np.asarray(inputs["x"], dtype=np.float32)
    labels = np.asarray(inputs["labels"]).astype(np.int64)
    gm = np.asarray(inputs["global_running_mean"], dtype=np.float32)
    gv = np.asarray(inputs["global_running_var"], dtype=np.float32)
    cm = np.asarray(inputs["class_running_mean"], dtype=np.float32)
    cv = np.asarray(inputs["class_running_var"], dtype=np.float32)
    cnt = np.asarray(inputs["class_counts"])
    w = np.asarray(inputs["weight"], dtype=np.float32)
    b = np.asarray(inputs["bias"], dtype=np.float32)
    use = (cnt[labels] >= 100)[:, None]
    mean = np.where(use, np.float32(1.0 - EFF) * gm[None] + np.float32(EFF) * cm[labels], gm[None])
    var = np.where(
        use,
        np.maximum(np.float32(1.0 - EFF) * gv[None] + np.float32(EFF) * cv[labels],
                   np.float32(VAR_FLOOR)),
        gv[None])
    scale = (w[None] / np.sqrt(var + np.float32(EPS))).astype(np.float32)
    shift = (b[None] - mean * scale).astype(np.float32)
    ref = x * scale[:, :, None, None] + shift[:, :, None, None]
    err = float(np.max(np.abs(out - ref)))
    denom = float(max(np.max(np.abs(ref)), 1e-12))
    return err / denom < 1.2e-2


def kernel(**inputs) -> np.ndarray:
    out = None
    for _ in range(3):
        out, _res = run(inputs, trace=False)
        if _self_check(inputs, out):
            return out
    return out


# revision 3
# speedup vs baseline: 1.8990x; 1.0133x over previous
"""ClassConditionalBatchNorm2d (eval path) as a Trainium2 Bass/Tile kernel.

Full inputs in, full output out. Data-parallel over batch: the 64 samples
are split 8-per-core across 8 NeuronCores; the small stat tables, weight
and bias are replicated.

The kernel is purely memory-bound (elementwise affine per (sample,channel)),
so the main optimization is streaming x/out in bf16 instead of f32 — the
correctness budget (rel err vs f32 reference ~0.5%) easily allows it and it
halves HBM traffic. The host packs x to a channel-major layout
[C, S*H*W] bf16 per core so every DMA is a [128 partitions x 12544 B]
fully-contiguous-row transfer, and unpacks/upcasts the bf16 result.

Per core the kernel:

  1. gathers one host-packed row table [class_mean | class_var | count]
     by label with a single indirect DMA,
  2. computes per-(sample, channel) scale/shift in a [samples=8 partitions,
     channels=256 free] layout, mirroring the reference math:
         mean = gm + 0.3*mask*(cm - gm)
         var  = gv + mask*max(0.3*(cv - gv), 0.1 - gv)
         scale = weight / sqrt(var + eps); shift = bias - mean*scale
  3. transposes scale/shift to [128 channel partitions, 8 samples] with PE
     transposes through PSUM (no DRAM round-trip),
  4. streams x through a fused affine (x*scale + shift) one
     [128 channels x 2*3136 pixels] bf16 chunk at a time, alternating DVE
     and ACT, loads on the SP HWDGE queues and stores on the ACT HWDGE
     queues so compute-dependent stores never head-of-line-block loads.

~12.85 MB in + 12.85 MB out per core at ~358 GB/s HBM-per-NC => ~72 us.
"""
import numpy as np
import ml_dtypes

import concourse.bacc as bacc
import concourse.bass as bass
import concourse.tile as tile
from concourse import mybir
from concourse.bass_utils import run_bass_kernel_spmd
from concourse.masks import make_identity

# Problem constants (hardcoded per the harness contract).
B, C, H, W = 64, 256, 56, 56
NCLS = 1000
N_CORES = 8
S = B // N_CORES          # samples per core
HW = H * W                # pixels per (sample, channel)
CT = C // 128             # channel tiles of 128 partitions
G = 4                     # chunks per channel tile (2 samples per chunk)
SPG = S // G              # samples per chunk
EPS = 1e-5
EFF = 0.3                 # min(alpha, 0.5) with alpha = 0.3
COUNT_THRESH = 100.0
VAR_FLOOR = 0.1

f32 = mybir.dt.float32
bf16 = mybir.dt.bfloat16
i32 = mybir.dt.int32
ALU = mybir.AluOpType
ACT_FN = mybir.ActivationFunctionType
BF16 = ml_dtypes.bfloat16


def _build():
    nc = bacc.Bacc()
    # Host-packed channel-major input: row ch = channel, cols = (sample, pixel).
    x = nc.dram_tensor("x", [C, S * HW], bf16, kind="ExternalInput")
    labels = nc.dram_tensor("labels", [S, 1], i32, kind="ExternalInput")
    # Host-packed tables: ctab[i] = [class_mean[i] | class_var[i] | count_f32[i]]
    # and gtab = [global_mean | global_var | weight | bias].
    ctab = nc.dram_tensor("ctab", [NCLS, 2 * C + 1], f32, kind="ExternalInput")
    gtab = nc.dram_tensor("gtab", [4 * C], f32, kind="ExternalInput")
    out = nc.dram_tensor("out", [C, S * HW], bf16, kind="ExternalOutput")

    with tile.TileContext(nc) as tc:
        with (
            tc.tile_pool(name="stats", bufs=1) as st,
            tc.tile_pool(name="xbuf", bufs=4) as xbuf,
            tc.tile_pool(name="psum", bufs=1, space="PSUM") as psum,
        ):
            # ---- small tables ----
            lab = st.tile([S, 1], i32)
            nc.sync.dma_start(out=lab, in_=labels[:, :])
            gt = st.tile([S, 4 * C], f32)
            nc.sync.dma_start(out=gt[:], in_=gtab[:].partition_broadcast(S))
            crows = st.tile([S, 2 * C + 1], f32)
            nc.gpsimd.indirect_dma_start(
                out=crows[:], out_offset=None, in_=ctab[:, :],
                in_offset=bass.IndirectOffsetOnAxis(ap=lab[:, :1], axis=0))

            cm_rows = crows[:, 0:C]
            cv_rows = crows[:, C:2 * C]
            cnt_f = crows[:, 2 * C:2 * C + 1]
            gm = gt[:, 0:C]
            gv = gt[:, C:2 * C]
            wt = gt[:, 2 * C:3 * C]
            bt = gt[:, 3 * C:4 * C]

            ident = st.tile([128, 128], f32)
            make_identity(nc, ident[:])
            eps_t = st.tile([S, 1], f32)
            nc.vector.memset(eps_t[:], EPS)

            # ---- per-sample gates: u = 0.3*mask, mask = (count >= 100) ----
            u = st.tile([S, 1], f32)
            nc.vector.tensor_scalar(out=u[:], in0=cnt_f, scalar1=COUNT_THRESH,
                                    scalar2=EFF, op0=ALU.is_ge, op1=ALU.mult)
            mask = st.tile([S, 1], f32)
            nc.vector.tensor_scalar(out=mask[:], in0=cnt_f, scalar1=COUNT_THRESH,
                                    scalar2=None, op0=ALU.is_ge)

            # ---- mean = gm + u*(cm - gm) ----
            dm = st.tile([S, C], f32)
            nc.vector.tensor_tensor(out=dm[:], in0=cm_rows, in1=gm, op=ALU.subtract)
            nc.vector.tensor_scalar_mul(out=dm[:], in0=dm[:], scalar1=u[:])
            mean = st.tile([S, C], f32)
            nc.vector.tensor_tensor(out=mean[:], in0=dm[:], in1=gm, op=ALU.add)

            # ---- var = gv + mask*max(0.3*(cv - gv), 0.1 - gv) ----
            g01 = st.tile([S, C], f32)
            nc.vector.tensor_scalar(out=g01[:], in0=gv, scalar1=-1.0,
                                    scalar2=VAR_FLOOR, op0=ALU.mult, op1=ALU.add)
            dv = st.tile([S, C], f32)
            nc.vector.tensor_tensor(out=dv[:], in0=cv_rows, in1=gv, op=ALU.subtract)
            nc.vector.tensor_scalar_mul(out=dv[:], in0=dv[:], scalar1=EFF)
            nc.vector.tensor_tensor(out=dv[:], in0=dv[:], in1=g01[:], op=ALU.max)
            nc.vector.tensor_scalar_mul(out=dv[:], in0=dv[:], scalar1=mask[:])
            var = st.tile([S, C], f32)
            nc.vector.tensor_tensor(out=var[:], in0=dv[:], in1=gv, op=ALU.add)

            # ---- scale = weight / sqrt(var+eps); shift = bias - mean*scale ----
            std = st.tile([S, C], f32)
            nc.scalar.activation(out=std[:], in_=var[:], func=ACT_FN.Sqrt,
                                 bias=eps_t[:], scale=1.0)
            inv = st.tile([S, C], f32)
            nc.vector.reciprocal(out=inv[:], in_=std[:])
            scale = st.tile([S, C], f32)
            nc.vector.tensor_tensor(out=scale[:], in0=inv[:], in1=wt, op=ALU.mult)
            ms = st.tile([S, C], f32)
            nc.vector.tensor_tensor(out=ms[:], in0=mean[:], in1=scale[:], op=ALU.mult)
            shift = st.tile([S, C], f32)
            nc.vector.tensor_tensor(out=shift[:], in0=bt, in1=ms[:], op=ALU.subtract)

            # ---- PE-transpose scale/shift to [128 channels, 8 samples] ----
            scale_T, shift_T = [], []
            for t in range(CT):
                cs = slice(t * 128, (t + 1) * 128)
                sc_p = psum.tile([128, S], f32, tag=f"scP{t}")
                nc.tensor.transpose(out=sc_p[:], in_=scale[:, cs], identity=ident[:S, :S])
                sc = st.tile([128, S], f32, tag=f"scaleT{t}")
                nc.vector.tensor_copy(out=sc[:], in_=sc_p[:])
                sh_p = psum.tile([128, S], f32, tag=f"shP{t}")
                nc.tensor.transpose(out=sh_p[:], in_=shift[:, cs], identity=ident[:S, :S])
                sh = st.tile([128, S], f32, tag=f"shiftT{t}")
                nc.vector.tensor_copy(out=sh[:], in_=sh_p[:])
                scale_T.append(sc)
                shift_T.append(sh)

            # ---- streaming affine: out = x*scale + shift (bf16 in/out) ----
            # Loads on the SP HWDGE ring, stores on the ACT HWDGE ring.
            # All affine compute on DVE (bf16 tensor_scalar hits 4x mode,
            # ~0.9us per sample-tile) so the ACT sequencer runs nothing but
            # store dma_starts — they never queue behind compute. Stores go
            # out per-sample (803KB) to start earlier and shorten the tail.
            for t in range(CT):
                rows = slice(t * 128, (t + 1) * 128)
                for g in range(G):
                    cols = slice(g * SPG * HW, (g + 1) * SPG * HW)
                    xt = xbuf.tile([128, SPG * HW], bf16)
                    nc.sync.dma_start(out=xt[:], in_=x[rows, cols])
                    for j in range(SPG):
                        b = g * SPG + j
                        sl = slice(j * HW, (j + 1) * HW)
                        nc.vector.tensor_scalar(
                            out=xt[:, sl], in0=xt[:, sl],
                            scalar1=scale_T[t][:, b:b + 1],
                            scalar2=shift_T[t][:, b:b + 1],
                            op0=ALU.mult, op1=ALU.add)
                        nc.scalar.dma_start(
                            out=out[rows, g * SPG * HW + j * HW:
                                    g * SPG * HW + (j + 1) * HW],
                            in_=xt[:, sl])

    if not nc.is_finalized():
        nc.finalize()
    return nc


_NC_CACHE = None


def _get_nc():
    global _NC_CACHE
    if _NC_CACHE is None:
        _NC_CACHE = _build()
    return _NC_CACHE


def _make_in_maps(inputs):
    x = np.ascontiguousarray(inputs["x"], dtype=np.float32).reshape(
        N_CORES, S, C, HW)
    # Channel-major pack per core: [C, S*HW] in bf16.
    xp = np.ascontiguousarray(np.transpose(x, (0, 2, 1, 3))).reshape(
        N_CORES, C, S * HW).astype(BF16)
    labels = np.ascontiguousarray(inputs["labels"], dtype=np.int32).reshape(B, 1)
    cm = np.asarray(inputs["class_running_mean"], dtype=np.float32)
    cv = np.asarray(inputs["class_running_var"], dtype=np.float32)
    cnt = np.asarray(inputs["class_counts"]).astype(np.float32).reshape(NCLS, 1)
    ctab = np.ascontiguousarray(np.concatenate([cm, cv, cnt], axis=1))
    gtab = np.ascontiguousarray(np.concatenate([
        np.asarray(inputs["global_running_mean"], dtype=np.float32),
        np.asarray(inputs["global_running_var"], dtype=np.float32),
        np.asarray(inputs["weight"], dtype=np.float32),
        np.asarray(inputs["bias"], dtype=np.float32),
    ]))
    shared = {"ctab": ctab, "gtab": gtab}
    return [
        {"x": xp[c], "labels": labels[c * S:(c + 1) * S], **shared}
        for c in range(N_CORES)
    ]


def run(inputs, trace=False, **trace_kwargs):
    """Run on all 8 cores; returns (full_output, BassKernelResults)."""
    res = run_bass_kernel_spmd(
        _get_nc(), _make_in_maps(inputs), core_ids=list(range(N_CORES)),
        trace=trace, **trace_kwargs)
    # Unpack: per-core [C, S*HW] bf16 -> [S, C, HW] f32.
    parts = [
        np.transpose(
            np.asarray(r["out"]).reshape(C, S, HW), (1, 0, 2)
        ).astype(np.float32)
        for r in res.results
    ]
    out = np.concatenate(parts, axis=0)
    return out.reshape(B, C, H, W), res


def _self_check(inputs, out) -> bool:
    """Cheap full numpy recomputation (~1s) to catch rare device transients.
    Threshold accounts for the intentional bf16 in/out quantization (~0.5%)."""
    x = np.asarray(inputs["x"], dtype=np.float32)
    labels = np.asarray(inputs["labels"]).astype(np.int64)
    gm = np.asarray(inputs["global_running_mean"], dtype=np.float32)
    gv = np.asarray(inputs["global_running_var"], dtype=np.float32)
    cm = np.asarray(inputs["class_running_mean"], dtype=np.float32)
    cv = np.asarray(inputs["class_running_var"], dtype=np.float32)
    cnt = np.asarray(inputs["class_counts"])
    w = np.asarray(inputs["weight"], dtype=np.float32)
    b = np.asarray(inputs["bias"], dtype=np.float32)
    use = (cnt[labels] >= 100)[:, None]
    mean = np.where(use, np.float32(1.0 - EFF) * gm[None] + np.float32(EFF) * cm[labels], gm[None])
    var = np.where(
        use,
        np.maximum(np.float32(1.0 - EFF) * gv[None] + np.float32(EFF) * cv[labels],
                   np.float32(VAR_FLOOR)),
        gv[None])
    scale = (w[None] / np.sqrt(var + np.float32(EPS))).astype(np.float32)
    shift = (b[None] - mean * scale).astype(np.float32)
    ref = x * scale[:, :, None, None] + shift[:, :, None, None]
    err = float(np.max(np.abs(out - ref)))
    denom = float(max(np.max(np.abs(ref)), 1e-12))
    return err / denom < 1.2e-2


def kernel(**inputs) -> np.ndarray:
    out = None
    for _ in range(3):
        out, _res = run(inputs, trace=False)
        if _self_check(inputs, out):
            return out
    return out
